# revision 14
# baseline (speedup 1.0000x reference)
"""ClusterAssignment (Student-t / vq codebook soft-assignment) Trainium2 kernel.

Math (ALPHA=1 => power=1):
    ns[n,k]  = max(||x_n - c_k||^2, 0) = ||x||^2 + ||c||^2 - 2 x.c   (>= ~430 here, relu moot)
    num[n,k] = 1 / (1 + ns[n,k])
    out[n,k] = num[n,k] / sum_k num[n,k]

Strategy: data-parallel over 8 NeuronCores (batch N=65536 -> 8192 rows/core,
centers replicated; no collectives). Per 128-row batch tile, the PE computes
P[n,k] = csq_k - 2 x.c in a 2-bank PSUM tile [128,1024] with 8 fp8 matmuls
(4 contraction chunks x 2 K-halves) and NOTHING else -- both norm terms ride
for free:

  - csq_k: contraction row 511 is stolen (x_511 dropped, ~1.8e-4 noise vs a
    2e-2 budget): bt row = 1.0, ct row = csq_k. No aug matmuls.
  - xsq_n: folded into the ScalarE ACT *scale* operand (exact f32):
        num'[n,k] = 1/(P*sc_n + 1) = b_n * num[n,k],  sc_n = 1/b_n = 1/(1+xsq_n)
    The b_n factor cancels in the final normalization, since the row-sum
    accumulator also scales by b_n. ACT input sits in [0.985, 1.019], the
    best-conditioned spot for the reciprocal table.

This puts the PE at the TRN2 roofline for this op: 4096 streamed columns per
tile, 1 column/cycle (DoubleRow/DoublePixel modes were measured/confirmed to
give no net stream speedup on this silicon), ~112us of gapless matmul per
core; everything else hides under it.

Epilogue per tile: ONE ScalarE Reciprocal pass (scale=sc_n, bias=1.0) reads
the full [128,1024] PSUM tile, writing num' as fp16 AND the f32 row-sum via
the ACT accumulator. Row-sum reciprocals are batched 8 tiles per DVE
instruction (4 for the last tiles, shortening the tail); out = num' * inv'
on DVE (fp16 4x mode); paired output DMAs (2 tiles -> one 4KB/partition
transfer) alternate between the Sync HWDGE queue and the otherwise-idle
GpSimd SWDGE queue so the output drain parallelizes. Host upcasts
fp16 -> f32.
"""

import sys

sys.path.insert(0, "/opt/trn_rl_repo")

from contextlib import ExitStack

import ml_dtypes
import numpy as np

import concourse.bass as bass
import concourse.mybir as mybir
import concourse.tile as tile
from concourse import bacc
from concourse.bass import ts
from concourse.bass_utils import run_bass_kernel_spmd

N, K, D = 65536, 512 * 2, 512  # K=1024
NCORES = 8
NS = N // NCORES  # 8192 rows per core
NT = NS // 128  # 64 tiles per core
NCH = D // 128  # 4 contraction chunks of 128
BF16 = mybir.dt.bfloat16
F32 = mybir.dt.float32
FP16 = mybir.dt.float16
FP8 = mybir.dt.float8e4  # e4m3 (TRN variant: max normal 240 -- our data is <6)
NP_FP8 = ml_dtypes.float8_e4m3


def _act_reciprocal(nc, out_ap, in_ap, scale_ap, accum_ap):
    """ScalarE activation out = 1/(in_*scale + 1) with row-sum accumulator.

    bass's activation() refuses ActivationFunctionType.Reciprocal because of
    known accuracy issues in the general case; on this kernel's input range
    ([~0.985, ~1.019] after the scale/bias affine) the error is at the fp16
    output rounding floor and the f32 accumulator is accurate to ~2e-6, so we
    emit the instruction directly. scale is a per-partition [128,1] f32 AP.
    """
    eng = nc.scalar
    ins = [
        eng.lower_ap(in_ap),
        mybir.ImmediateValue(dtype=F32, value=1.0),  # bias
        eng.lower_ap(scale_ap),  # scale = 1/(1+xsq_n)
        mybir.ImmediateValue(dtype=F32, value=0.0),  # alpha
    ]
    outs = [eng.lower_ap(out_ap), eng.lower_ap(accum_ap)]
    return eng.add_instruction(
        mybir.InstActivation(
            name=nc.get_next_instruction_name(),
            func=mybir.ActivationFunctionType.Reciprocal,
            ins=ins,
            outs=outs,
        )
    )


def build_bass():
    nc = bacc.Bacc("TRN2", target_bir_lowering=False, debug=False)
    bt = nc.declare_dram_parameter("bt", [128, NT, NCH, 128], FP8, isOutput=False)
    ct = nc.declare_dram_parameter("ct", [128, NCH, K], FP8, isOutput=False)
    sc = nc.declare_dram_parameter("sc", [128, NT], F32, isOutput=False)
    # out[u, w, p, k] = row u*256 + w*128 + p -> host reshapes to [NS, K]
    out = nc.declare_dram_parameter("out", [NT // 2, 2, 128, K], FP16, isOutput=True)

    # tiles per row-sum reciprocal batch: one DVE inv instruction per group;
    # smaller final groups release the last output DMAs sooner
    groups = [8] * 6 + [4, 4, 2, 2, 2, 2]
    assert sum(groups) == NT

    with tile.TileContext(nc) as tc, ExitStack() as ctx:
        singles = ctx.enter_context(tc.tile_pool(name="singles", bufs=1))
        bpool = ctx.enter_context(tc.tile_pool(name="bt", bufs=3))
        npool = ctx.enter_context(tc.tile_pool(name="num", bufs=18))
        opool = ctx.enter_context(tc.tile_pool(name="outp", bufs=4))
        spool = ctx.enter_context(tc.tile_pool(name="small", bufs=6))
        psum = ctx.enter_context(tc.tile_pool(name="psum", bufs=3, space="PSUM"))

        # ct first on the Sync HWDGE queue (lowest first-transfer latency;
        # the queue fans out to ~15 DMA engines so ct/bt0/sc parallelize)
        ct_sb = singles.tile([128, NCH, K], FP8)
        nc.sync.dma_start(out=ct_sb[:], in_=ct[:])
        sc_sb = singles.tile([128, NT], F32)
        nc.sync.dma_start(out=sc_sb[:], in_=sc[:])

        t = 0
        opair = 0
        for gsz in groups:
            rs = spool.tile([128, gsz], F32)
            nums = []
            for j in range(gsz):
                if t % 4 == 0:  # 4 tiles per input DMA: 2KB per partition line
                    bt_t = bpool.tile([128, 4, NCH, 128], FP8)
                    nc.sync.dma_start(out=bt_t[:], in_=bt[:, ts(t // 4, 4)])
                w = t % 4
                ps = psum.tile([128, K], F32)  # 2 banks; each matmul hits one
                # interleave the two kh accumulation groups so every
                # LDWEIGHTS hides under the previous matmul's stream
                for c in range(NCH):
                    for kh in range(2):
                        nc.tensor.matmul(
                            ps[:, ts(kh, 512)],
                            lhsT=bt_t[:, w, c],
                            rhs=ct_sb[:, c, ts(kh, 512)],
                            start=(c == 0),
                            stop=(c == NCH - 1),
                            skip_group_check=True,
                        )
                num = npool.tile([128, K], FP16)
                _act_reciprocal(nc, num[:], ps[:], sc_sb[:, t : t + 1], rs[:, j : j + 1])
                nums.append(num)
                t += 1
            inv = spool.tile([128, gsz], F32)
            nc.vector.reciprocal(out=inv[:], in_=rs[:])
            for pj in range(gsz // 2):  # paired output DMAs: 4KB/partition
                o2 = opool.tile([128, 2, K], FP16)
                for w2 in range(2):
                    j = 2 * pj + w2
                    nc.vector.tensor_scalar_mul(
                        o2[:, w2], nums[j][:], inv[:, j : j + 1]
                    )
                eng = nc.gpsimd if opair % 2 == 0 else nc.sync
                eng.dma_start(out=out[opair].rearrange("a b c -> b a c"), in_=o2[:])
                opair += 1
    nc.finalize()
    return nc


_NC_CACHE = None


def _get_nc():
    global _NC_CACHE
    if _NC_CACHE is None:
        _NC_CACHE = build_bass()
    return _NC_CACHE


def prepare_inputs(batch: np.ndarray, cluster_centers: np.ndarray):
    """Host-side shard + layout. Returns in_maps for run_bass_kernel_spmd."""
    assert batch.shape == (N, D) and cluster_centers.shape == (K, D)
    b32 = batch.astype(np.float32, copy=False)
    c32 = cluster_centers.astype(np.float32, copy=False)
    xsq = np.einsum("nd,nd->n", b32, b32)  # [N]
    csq = np.einsum("kd,kd->k", c32, c32)  # [K]

    # ct[p, c, k] = -2 * centers[k, c*128+p]; stolen row 511 carries csq_k
    cmod = -2.0 * c32  # [K, D]
    ctf = cmod.T.reshape(NCH, 128, K).transpose(1, 0, 2)  # [p, c, k]
    ctf = np.ascontiguousarray(ctf, dtype=NP_FP8)
    ctf[127, NCH - 1, :] = csq.astype(NP_FP8)

    in_maps = []
    for i in range(NCORES):
        shard = b32[i * NS : (i + 1) * NS]
        # bt[p, t, c, j] = shard[t*128+j, c*128+p]; stolen row 511 = 1.0
        bts = shard.reshape(NT, 128, NCH, 128).transpose(3, 0, 2, 1)
        bts = np.ascontiguousarray(bts, dtype=NP_FP8)
        bts[127, :, NCH - 1, :] = 1.0
        # sc[p, t] = 1/(1 + xsq[t*128+p]), exact f32
        scs = 1.0 / (
            1.0 + xsq[i * NS : (i + 1) * NS].reshape(NT, 128).T
        )
        scs = np.ascontiguousarray(scs, dtype=np.float32)
        in_maps.append({"bt": bts, "ct": ctf, "sc": scs})
    return in_maps


def kernel(batch: np.ndarray, cluster_centers: np.ndarray, _trace=False) -> np.ndarray:
    nc = _get_nc()
    in_maps = prepare_inputs(batch, cluster_centers)
    res = run_bass_kernel_spmd(nc, in_maps, list(range(NCORES)), trace=_trace)
    out = np.concatenate(
        [
            res.results[i]["out"].reshape(NS, K).astype(np.float32)
            for i in range(NCORES)
        ],
        axis=0,
    )
    if _trace:
        return out, res
    return out


# revision 15
# speedup vs baseline: 1.0030x; 1.0030x over previous
"""ClusterAssignment (Student-t / vq codebook soft-assignment) Trainium2 kernel.

Math (ALPHA=1 => power=1):
    ns[n,k]  = max(||x_n - c_k||^2, 0) = ||x||^2 + ||c||^2 - 2 x.c   (>= ~430 here, relu moot)
    num[n,k] = 1 / (1 + ns[n,k])
    out[n,k] = num[n,k] / sum_k num[n,k]

Strategy: data-parallel over 8 NeuronCores (batch N=65536 -> 8192 rows/core,
centers replicated; no collectives). Per 128-row batch tile, the PE computes
P[n,k] = csq_k - 2 x.c in a 2-bank PSUM tile [128,1024] with 8 fp8 matmuls
(4 contraction chunks x 2 K-halves) and NOTHING else -- both norm terms ride
for free:

  - csq_k: contraction row 511 is stolen (x_511 dropped, ~1.8e-4 noise vs a
    2e-2 budget): bt row = 1.0, ct row = csq_k. No aug matmuls.
  - xsq_n: folded into the ScalarE ACT *scale* operand (exact f32):
        num'[n,k] = 1/(P*sc_n + 1) = b_n * num[n,k],  sc_n = 1/b_n = 1/(1+xsq_n)
    The b_n factor cancels in the final normalization, since the row-sum
    accumulator also scales by b_n. ACT input sits in [0.985, 1.019], the
    best-conditioned spot for the reciprocal table.

This puts the PE at the TRN2 roofline for this op: 4096 streamed columns per
tile, 1 column/cycle (DoubleRow/DoublePixel modes were measured/confirmed to
give no net stream speedup on this silicon), ~112us of gapless matmul per
core; everything else hides under it.

Epilogue per tile: ONE ScalarE Reciprocal pass (scale=sc_n, bias=1.0) reads
the full [128,1024] PSUM tile, writing num' as fp16 AND the f32 row-sum via
the ACT accumulator. Row-sum reciprocals are batched 8 tiles per DVE
instruction (4 for the last tiles, shortening the tail); out = num' * inv'
on DVE (fp16 4x mode); paired output DMAs (2 tiles -> one 4KB/partition
transfer) alternate between the Sync HWDGE queue and the otherwise-idle
GpSimd SWDGE queue so the output drain parallelizes. Host upcasts
fp16 -> f32.
"""

import sys

sys.path.insert(0, "/opt/trn_rl_repo")

from contextlib import ExitStack

import ml_dtypes
import numpy as np

import concourse.bass as bass
import concourse.mybir as mybir
import concourse.tile as tile
from concourse import bacc
from concourse.bass import ts
from concourse.bass_utils import run_bass_kernel_spmd

N, K, D = 65536, 512 * 2, 512  # K=1024
NCORES = 8
NS = N // NCORES  # 8192 rows per core
NT = NS // 128  # 64 tiles per core
NCH = D // 128  # 4 contraction chunks of 128
BF16 = mybir.dt.bfloat16
F32 = mybir.dt.float32
FP16 = mybir.dt.float16
FP8 = mybir.dt.float8e4  # e4m3 (TRN variant: max normal 240 -- our data is <6)
NP_FP8 = ml_dtypes.float8_e4m3


def _act_reciprocal(nc, out_ap, in_ap, scale_ap, accum_ap):
    """ScalarE activation out = 1/(in_*scale + 1) with row-sum accumulator.

    bass's activation() refuses ActivationFunctionType.Reciprocal because of
    known accuracy issues in the general case; on this kernel's input range
    ([~0.985, ~1.019] after the scale/bias affine) the error is at the fp16
    output rounding floor and the f32 accumulator is accurate to ~2e-6, so we
    emit the instruction directly. scale is a per-partition [128,1] f32 AP.
    """
    eng = nc.scalar
    ins = [
        eng.lower_ap(in_ap),
        mybir.ImmediateValue(dtype=F32, value=1.0),  # bias
        eng.lower_ap(scale_ap),  # scale = 1/(1+xsq_n)
        mybir.ImmediateValue(dtype=F32, value=0.0),  # alpha
    ]
    outs = [eng.lower_ap(out_ap), eng.lower_ap(accum_ap)]
    return eng.add_instruction(
        mybir.InstActivation(
            name=nc.get_next_instruction_name(),
            func=mybir.ActivationFunctionType.Reciprocal,
            ins=ins,
            outs=outs,
        )
    )


def build_bass():
    nc = bacc.Bacc("TRN2", target_bir_lowering=False, debug=False)
    bt = nc.declare_dram_parameter("bt", [128, NT, NCH, 128], FP8, isOutput=False)
    ct = nc.declare_dram_parameter("ct", [128, NCH, K], FP8, isOutput=False)
    sc = nc.declare_dram_parameter("sc", [128, NT], F32, isOutput=False)
    # out[u, w, p, k] = row u*256 + w*128 + p -> host reshapes to [NS, K]
    out = nc.declare_dram_parameter("out", [NT // 2, 2, 128, K], FP16, isOutput=True)

    # tiles per row-sum reciprocal batch: one DVE inv instruction per group;
    # smaller final groups release the last output DMAs sooner
    groups = [8] * 6 + [4, 4, 2, 2, 2, 2]
    assert sum(groups) == NT

    with tile.TileContext(nc) as tc, ExitStack() as ctx:
        singles = ctx.enter_context(tc.tile_pool(name="singles", bufs=1))
        bpool = ctx.enter_context(tc.tile_pool(name="bt", bufs=3))
        npool = ctx.enter_context(tc.tile_pool(name="num", bufs=18))
        opool = ctx.enter_context(tc.tile_pool(name="outp", bufs=4))
        spool = ctx.enter_context(tc.tile_pool(name="small", bufs=6))
        psum = ctx.enter_context(tc.tile_pool(name="psum", bufs=3, space="PSUM"))

        # ct first on the Sync HWDGE queue (lowest first-transfer latency;
        # the queue fans out to ~15 DMA engines so ct/bt0/sc parallelize)
        ct_sb = singles.tile([128, NCH, K], FP8)
        nc.sync.dma_start(out=ct_sb[:], in_=ct[:])
        sc_sb = singles.tile([128, NT], F32)
        nc.sync.dma_start(out=sc_sb[:], in_=sc[:])

        # PE p-state warm-up: the PE needs ~3us of continuous work to reach
        # full clock. Fill the input-DMA wait with dummy matmuls on zeroed
        # scratch so the real stream starts warm instead of ramping.
        warm = singles.tile([128, 512], FP8)
        nc.gpsimd.memset(warm[:], 0)
        wps = ctx.enter_context(tc.tile_pool(name="warmps", bufs=1, space="PSUM"))
        wp = wps.tile([128, 512], F32)
        for _ in range(24):
            nc.tensor.matmul(
                wp[:],
                lhsT=warm[:, :128],
                rhs=warm[:],
                start=True,
                stop=True,
                skip_group_check=True,
            )

        t = 0
        opair = 0
        for gsz in groups:
            rs = spool.tile([128, gsz], F32)
            nums = []
            for j in range(gsz):
                if t % 4 == 0:  # 4 tiles per input DMA: 2KB per partition line
                    bt_t = bpool.tile([128, 4, NCH, 128], FP8)
                    nc.sync.dma_start(out=bt_t[:], in_=bt[:, ts(t // 4, 4)])
                w = t % 4
                ps = psum.tile([128, K], F32)  # 2 banks; each matmul hits one
                # interleave the two kh accumulation groups so every
                # LDWEIGHTS hides under the previous matmul's stream
                for c in range(NCH):
                    for kh in range(2):
                        nc.tensor.matmul(
                            ps[:, ts(kh, 512)],
                            lhsT=bt_t[:, w, c],
                            rhs=ct_sb[:, c, ts(kh, 512)],
                            start=(c == 0),
                            stop=(c == NCH - 1),
                            skip_group_check=True,
                        )
                num = npool.tile([128, K], FP16)
                _act_reciprocal(nc, num[:], ps[:], sc_sb[:, t : t + 1], rs[:, j : j + 1])
                nums.append(num)
                t += 1
            inv = spool.tile([128, gsz], F32)
            nc.vector.reciprocal(out=inv[:], in_=rs[:])
            for pj in range(gsz // 2):  # paired output DMAs: 4KB/partition
                o2 = opool.tile([128, 2, K], FP16)
                for w2 in range(2):
                    j = 2 * pj + w2
                    nc.vector.tensor_scalar_mul(
                        o2[:, w2], nums[j][:], inv[:, j : j + 1]
                    )
                eng = nc.gpsimd if opair % 2 == 0 else nc.sync
                eng.dma_start(out=out[opair].rearrange("a b c -> b a c"), in_=o2[:])
                opair += 1
    nc.finalize()
    return nc


_NC_CACHE = None


def _get_nc():
    global _NC_CACHE
    if _NC_CACHE is None:
        _NC_CACHE = build_bass()
    return _NC_CACHE


def prepare_inputs(batch: np.ndarray, cluster_centers: np.ndarray):
    """Host-side shard + layout. Returns in_maps for run_bass_kernel_spmd."""
    assert batch.shape == (N, D) and cluster_centers.shape == (K, D)
    b32 = batch.astype(np.float32, copy=False)
    c32 = cluster_centers.astype(np.float32, copy=False)
    xsq = np.einsum("nd,nd->n", b32, b32)  # [N]
    csq = np.einsum("kd,kd->k", c32, c32)  # [K]

    # ct[p, c, k] = -2 * centers[k, c*128+p]; stolen row 511 carries csq_k
    cmod = -2.0 * c32  # [K, D]
    ctf = cmod.T.reshape(NCH, 128, K).transpose(1, 0, 2)  # [p, c, k]
    ctf = np.ascontiguousarray(ctf, dtype=NP_FP8)
    ctf[127, NCH - 1, :] = csq.astype(NP_FP8)

    in_maps = []
    for i in range(NCORES):
        shard = b32[i * NS : (i + 1) * NS]
        # bt[p, t, c, j] = shard[t*128+j, c*128+p]; stolen row 511 = 1.0
        bts = shard.reshape(NT, 128, NCH, 128).transpose(3, 0, 2, 1)
        bts = np.ascontiguousarray(bts, dtype=NP_FP8)
        bts[127, :, NCH - 1, :] = 1.0
        # sc[p, t] = 1/(1 + xsq[t*128+p]), exact f32
        scs = 1.0 / (
            1.0 + xsq[i * NS : (i + 1) * NS].reshape(NT, 128).T
        )
        scs = np.ascontiguousarray(scs, dtype=np.float32)
        in_maps.append({"bt": bts, "ct": ctf, "sc": scs})
    return in_maps


def kernel(batch: np.ndarray, cluster_centers: np.ndarray, _trace=False) -> np.ndarray:
    nc = _get_nc()
    in_maps = prepare_inputs(batch, cluster_centers)
    res = run_bass_kernel_spmd(nc, in_maps, list(range(NCORES)), trace=_trace)
    out = np.concatenate(
        [
            res.results[i]["out"].reshape(NS, K).astype(np.float32)
            for i in range(NCORES)
        ],
        axis=0,
    )
    if _trace:
        return out, res
    return out


# revision 16
# speedup vs baseline: 1.0208x; 1.0177x over previous
"""ClusterAssignment (Student-t / vq codebook soft-assignment) Trainium2 kernel.

Math (ALPHA=1 => power=1):
    ns[n,k]  = max(||x_n - c_k||^2, 0) = ||x||^2 + ||c||^2 - 2 x.c   (>= ~430 here, relu moot)
    num[n,k] = 1 / (1 + ns[n,k])
    out[n,k] = num[n,k] / sum_k num[n,k]

Strategy: data-parallel over 8 NeuronCores (batch N=65536 -> 8192 rows/core,
centers replicated; no collectives). Per 128-row batch tile, the PE computes
P[n,k] = csq_k - 2 x.c in a 2-bank PSUM tile [128,1024] with 8 fp8 matmuls
(4 contraction chunks x 2 K-halves) and NOTHING else -- both norm terms ride
for free:

  - csq_k: contraction row 511 is stolen (x_511 dropped, ~1.8e-4 noise vs a
    2e-2 budget): bt row = 1.0, ct row = csq_k. No aug matmuls.
  - xsq_n: folded into the ScalarE ACT *scale* operand (exact f32):
        num'[n,k] = 1/(P*sc_n + 1) = b_n * num[n,k],  sc_n = 1/b_n = 1/(1+xsq_n)
    The b_n factor cancels in the final normalization, since the row-sum
    accumulator also scales by b_n. ACT input sits in [0.985, 1.019], the
    best-conditioned spot for the reciprocal table.

This puts the PE at the TRN2 roofline for this op: 4096 streamed columns per
tile, 1 column/cycle (DoubleRow/DoublePixel modes were measured/confirmed to
give no net stream speedup on this silicon), ~112us of gapless matmul per
core; everything else hides under it.

Epilogue per tile: ONE ScalarE Reciprocal pass (scale=sc_n, bias=1.0) reads
the full [128,1024] PSUM tile, writing num' as fp16 AND the f32 row-sum via
the ACT accumulator. Row-sum reciprocals are batched 8 tiles per DVE
instruction (4 for the last tiles, shortening the tail); out = num' * inv'
on DVE (fp16 4x mode); paired output DMAs (2 tiles -> one 4KB/partition
transfer) alternate between the Sync HWDGE queue and the otherwise-idle
GpSimd SWDGE queue so the output drain parallelizes. Host upcasts
fp16 -> f32.
"""

import sys

sys.path.insert(0, "/opt/trn_rl_repo")

from contextlib import ExitStack

import ml_dtypes
import numpy as np

import concourse.bass as bass
import concourse.mybir as mybir
import concourse.tile as tile
from concourse import bacc
from concourse.bass import ts
from concourse.bass_utils import run_bass_kernel_spmd

N, K, D = 65536, 512 * 2, 512  # K=1024
NCORES = 8
NS = N // NCORES  # 8192 rows per core
NT = NS // 128  # 64 tiles per core
NCH = D // 128  # 4 contraction chunks of 128
BF16 = mybir.dt.bfloat16
F32 = mybir.dt.float32
FP16 = mybir.dt.float16
FP8 = mybir.dt.float8e4  # e4m3 (TRN variant: max normal 240 -- our data is <6)
NP_FP8 = ml_dtypes.float8_e4m3


def _act_reciprocal(nc, out_ap, in_ap, scale_ap, accum_ap):
    """ScalarE activation out = 1/(in_*scale + 1) with row-sum accumulator.

    bass's activation() refuses ActivationFunctionType.Reciprocal because of
    known accuracy issues in the general case; on this kernel's input range
    ([~0.985, ~1.019] after the scale/bias affine) the error is at the fp16
    output rounding floor and the f32 accumulator is accurate to ~2e-6, so we
    emit the instruction directly. scale is a per-partition [128,1] f32 AP.
    """
    eng = nc.scalar
    ins = [
        eng.lower_ap(in_ap),
        mybir.ImmediateValue(dtype=F32, value=1.0),  # bias
        eng.lower_ap(scale_ap),  # scale = 1/(1+xsq_n)
        mybir.ImmediateValue(dtype=F32, value=0.0),  # alpha
    ]
    outs = [eng.lower_ap(out_ap), eng.lower_ap(accum_ap)]
    return eng.add_instruction(
        mybir.InstActivation(
            name=nc.get_next_instruction_name(),
            func=mybir.ActivationFunctionType.Reciprocal,
            ins=ins,
            outs=outs,
        )
    )


def build_bass():
    nc = bacc.Bacc("TRN2", target_bir_lowering=False, debug=False)
    bt = nc.declare_dram_parameter("bt", [128, NT, NCH, 128], FP8, isOutput=False)
    ct = nc.declare_dram_parameter("ct", [128, NCH, K], FP8, isOutput=False)
    sc = nc.declare_dram_parameter("sc", [128, NT], F32, isOutput=False)
    # out[u, w, p, k] = row u*256 + w*128 + p -> host reshapes to [NS, K]
    out = nc.declare_dram_parameter("out", [NT // 2, 2, 128, K], FP16, isOutput=True)

    # tiles per row-sum reciprocal batch: one DVE inv instruction per group;
    # smaller final groups release the last output DMAs sooner
    groups = [8] * 6 + [4, 4, 2, 2, 2, 2]
    assert sum(groups) == NT

    with tile.TileContext(nc) as tc, ExitStack() as ctx:
        singles = ctx.enter_context(tc.tile_pool(name="singles", bufs=1))
        bpool = ctx.enter_context(tc.tile_pool(name="bt", bufs=3))
        npool = ctx.enter_context(tc.tile_pool(name="num", bufs=18))
        opool = ctx.enter_context(tc.tile_pool(name="outp", bufs=4))
        spool = ctx.enter_context(tc.tile_pool(name="small", bufs=6))
        psum = ctx.enter_context(tc.tile_pool(name="psum", bufs=3, space="PSUM"))

        # ct first on the Sync HWDGE queue (lowest first-transfer latency;
        # the queue fans out to ~15 DMA engines so ct/bt0/sc parallelize)
        ct_sb = singles.tile([128, NCH, K], FP8)
        nc.sync.dma_start(out=ct_sb[:], in_=ct[:])
        sc_sb = singles.tile([128, NT], F32)
        nc.sync.dma_start(out=sc_sb[:], in_=sc[:])

        # PE p-state warm-up: the PE needs ~3us of continuous work to reach
        # full clock. Fill the input-DMA wait with dummy matmuls on zeroed
        # scratch so the real stream starts warm instead of ramping.
        warm = singles.tile([128, 512], FP8)
        nc.gpsimd.memset(warm[:], 0)
        wps = ctx.enter_context(tc.tile_pool(name="warmps", bufs=1, space="PSUM"))
        wp = wps.tile([128, 512], F32)
        for _ in range(8):
            nc.tensor.matmul(
                wp[:],
                lhsT=warm[:, :128],
                rhs=warm[:],
                start=True,
                stop=True,
                skip_group_check=True,
            )

        t = 0
        opair = 0
        for gsz in groups:
            rs = spool.tile([128, gsz], F32)
            nums = []
            for j in range(gsz):
                if t % 4 == 0:  # 4 tiles per input DMA: 2KB per partition line
                    bt_t = bpool.tile([128, 4, NCH, 128], FP8)
                    nc.sync.dma_start(out=bt_t[:], in_=bt[:, ts(t // 4, 4)])
                w = t % 4
                ps = psum.tile([128, K], F32)  # 2 banks; each matmul hits one
                # interleave the two kh accumulation groups so every
                # LDWEIGHTS hides under the previous matmul's stream
                for c in range(NCH):
                    for kh in range(2):
                        nc.tensor.matmul(
                            ps[:, ts(kh, 512)],
                            lhsT=bt_t[:, w, c],
                            rhs=ct_sb[:, c, ts(kh, 512)],
                            start=(c == 0),
                            stop=(c == NCH - 1),
                            skip_group_check=True,
                        )
                num = npool.tile([128, K], FP16)
                _act_reciprocal(nc, num[:], ps[:], sc_sb[:, t : t + 1], rs[:, j : j + 1])
                nums.append(num)
                t += 1
            inv = spool.tile([128, gsz], F32)
            nc.vector.reciprocal(out=inv[:], in_=rs[:])
            for pj in range(gsz // 2):  # paired output DMAs: 4KB/partition
                o2 = opool.tile([128, 2, K], FP16)
                for w2 in range(2):
                    j = 2 * pj + w2
                    nc.vector.tensor_scalar_mul(
                        o2[:, w2], nums[j][:], inv[:, j : j + 1]
                    )
                eng = nc.gpsimd if opair % 2 == 0 else nc.sync
                eng.dma_start(out=out[opair].rearrange("a b c -> b a c"), in_=o2[:])
                opair += 1
    nc.finalize()
    return nc


_NC_CACHE = None


def _get_nc():
    global _NC_CACHE
    if _NC_CACHE is None:
        _NC_CACHE = build_bass()
    return _NC_CACHE


def prepare_inputs(batch: np.ndarray, cluster_centers: np.ndarray):
    """Host-side shard + layout. Returns in_maps for run_bass_kernel_spmd."""
    assert batch.shape == (N, D) and cluster_centers.shape == (K, D)
    b32 = batch.astype(np.float32, copy=False)
    c32 = cluster_centers.astype(np.float32, copy=False)
    xsq = np.einsum("nd,nd->n", b32, b32)  # [N]
    csq = np.einsum("kd,kd->k", c32, c32)  # [K]

    # ct[p, c, k] = -2 * centers[k, c*128+p]; stolen row 511 carries csq_k
    cmod = -2.0 * c32  # [K, D]
    ctf = cmod.T.reshape(NCH, 128, K).transpose(1, 0, 2)  # [p, c, k]
    ctf = np.ascontiguousarray(ctf, dtype=NP_FP8)
    ctf[127, NCH - 1, :] = csq.astype(NP_FP8)

    in_maps = []
    for i in range(NCORES):
        shard = b32[i * NS : (i + 1) * NS]
        # bt[p, t, c, j] = shard[t*128+j, c*128+p]; stolen row 511 = 1.0
        bts = shard.reshape(NT, 128, NCH, 128).transpose(3, 0, 2, 1)
        bts = np.ascontiguousarray(bts, dtype=NP_FP8)
        bts[127, :, NCH - 1, :] = 1.0
        # sc[p, t] = 1/(1 + xsq[t*128+p]), exact f32
        scs = 1.0 / (
            1.0 + xsq[i * NS : (i + 1) * NS].reshape(NT, 128).T
        )
        scs = np.ascontiguousarray(scs, dtype=np.float32)
        in_maps.append({"bt": bts, "ct": ctf, "sc": scs})
    return in_maps


def kernel(batch: np.ndarray, cluster_centers: np.ndarray, _trace=False) -> np.ndarray:
    nc = _get_nc()
    in_maps = prepare_inputs(batch, cluster_centers)
    res = run_bass_kernel_spmd(nc, in_maps, list(range(NCORES)), trace=_trace)
    out = np.concatenate(
        [
            res.results[i]["out"].reshape(NS, K).astype(np.float32)
            for i in range(NCORES)
        ],
        axis=0,
    )
    if _trace:
        return out, res
    return out


# revision 17
# speedup vs baseline: 1.0229x; 1.0020x over previous
"""ClusterAssignment (Student-t / vq codebook soft-assignment) Trainium2 kernel.

Math (ALPHA=1 => power=1):
    ns[n,k]  = max(||x_n - c_k||^2, 0) = ||x||^2 + ||c||^2 - 2 x.c   (>= ~430 here, relu moot)
    num[n,k] = 1 / (1 + ns[n,k])
    out[n,k] = num[n,k] / sum_k num[n,k]

Strategy: data-parallel over 8 NeuronCores (batch N=65536 -> 8192 rows/core,
centers replicated; no collectives). Per 128-row batch tile, the PE computes
P[n,k] = csq_k - 2 x.c in a 2-bank PSUM tile [128,1024] with 8 fp8 matmuls
(4 contraction chunks x 2 K-halves) and NOTHING else -- both norm terms ride
for free:

  - csq_k: contraction row 511 is stolen (x_511 dropped, ~1.8e-4 noise vs a
    2e-2 budget): bt row = 1.0, ct row = csq_k. No aug matmuls.
  - xsq_n: folded into the ScalarE ACT *scale* operand (exact f32):
        num'[n,k] = 1/(P*sc_n + 1) = b_n * num[n,k],  sc_n = 1/b_n = 1/(1+xsq_n)
    The b_n factor cancels in the final normalization, since the row-sum
    accumulator also scales by b_n. ACT input sits in [0.985, 1.019], the
    best-conditioned spot for the reciprocal table.

This puts the PE at the TRN2 roofline for this op: 4096 streamed columns per
tile, 1 column/cycle (DoubleRow/DoublePixel modes were measured/confirmed to
give no net stream speedup on this silicon), ~112us of gapless matmul per
core; everything else hides under it.

Epilogue per tile: ONE ScalarE Reciprocal pass (scale=sc_n, bias=1.0) reads
the full [128,1024] PSUM tile, writing num' as fp16 AND the f32 row-sum via
the ACT accumulator. Row-sum reciprocals are batched 8 tiles per DVE
instruction (4 for the last tiles, shortening the tail); out = num' * inv'
on DVE (fp16 4x mode); paired output DMAs (2 tiles -> one 4KB/partition
transfer) alternate between the Sync HWDGE queue and the otherwise-idle
GpSimd SWDGE queue so the output drain parallelizes. Host upcasts
fp16 -> f32.
"""

import sys

sys.path.insert(0, "/opt/trn_rl_repo")

from contextlib import ExitStack

import ml_dtypes
import numpy as np

import concourse.bass as bass
import concourse.mybir as mybir
import concourse.tile as tile
from concourse import bacc
from concourse.bass import ts
from concourse.bass_utils import run_bass_kernel_spmd

N, K, D = 65536, 512 * 2, 512  # K=1024
NCORES = 8
NS = N // NCORES  # 8192 rows per core
NT = NS // 128  # 64 tiles per core
NCH = D // 128  # 4 contraction chunks of 128
BF16 = mybir.dt.bfloat16
F32 = mybir.dt.float32
FP16 = mybir.dt.float16
FP8 = mybir.dt.float8e4  # e4m3 (TRN variant: max normal 240 -- our data is <6)
NP_FP8 = ml_dtypes.float8_e4m3


def _act_reciprocal(nc, out_ap, in_ap, scale_ap, accum_ap):
    """ScalarE activation out = 1/(in_*scale + 1) with row-sum accumulator.

    bass's activation() refuses ActivationFunctionType.Reciprocal because of
    known accuracy issues in the general case; on this kernel's input range
    ([~0.985, ~1.019] after the scale/bias affine) the error is at the fp16
    output rounding floor and the f32 accumulator is accurate to ~2e-6, so we
    emit the instruction directly. scale is a per-partition [128,1] f32 AP.
    """
    eng = nc.scalar
    ins = [
        eng.lower_ap(in_ap),
        mybir.ImmediateValue(dtype=F32, value=1.0),  # bias
        eng.lower_ap(scale_ap),  # scale = 1/(1+xsq_n)
        mybir.ImmediateValue(dtype=F32, value=0.0),  # alpha
    ]
    outs = [eng.lower_ap(out_ap), eng.lower_ap(accum_ap)]
    return eng.add_instruction(
        mybir.InstActivation(
            name=nc.get_next_instruction_name(),
            func=mybir.ActivationFunctionType.Reciprocal,
            ins=ins,
            outs=outs,
        )
    )


def build_bass():
    nc = bacc.Bacc("TRN2", target_bir_lowering=False, debug=False)
    bt = nc.declare_dram_parameter("bt", [128, NT, NCH, 128], FP8, isOutput=False)
    ct = nc.declare_dram_parameter("ct", [128, NCH, K], FP8, isOutput=False)
    sc = nc.declare_dram_parameter("sc", [128, NT], F32, isOutput=False)
    # out[u, w, p, k] = row u*256 + w*128 + p -> host reshapes to [NS, K]
    out = nc.declare_dram_parameter("out", [NT // 2, 2, 128, K], FP16, isOutput=True)

    # tiles per row-sum reciprocal batch: one DVE inv instruction per group;
    # smaller final groups release the last output DMAs sooner
    groups = [8] * 6 + [4, 4, 2, 2, 2, 2]
    assert sum(groups) == NT

    with tile.TileContext(nc) as tc, ExitStack() as ctx:
        singles = ctx.enter_context(tc.tile_pool(name="singles", bufs=1))
        bpool = ctx.enter_context(tc.tile_pool(name="bt", bufs=3))
        npool = ctx.enter_context(tc.tile_pool(name="num", bufs=18))
        opool = ctx.enter_context(tc.tile_pool(name="outp", bufs=4))
        spool = ctx.enter_context(tc.tile_pool(name="small", bufs=6))
        psum = ctx.enter_context(tc.tile_pool(name="psum", bufs=3, space="PSUM"))

        # ct first on the Sync HWDGE queue (lowest first-transfer latency;
        # the queue fans out to ~15 DMA engines so ct/bt0/sc parallelize)
        ct_sb = singles.tile([128, NCH, K], FP8)
        nc.sync.dma_start(out=ct_sb[:], in_=ct[:])
        sc_sb = singles.tile([128, NT], F32)
        nc.sync.dma_start(out=sc_sb[:], in_=sc[:])

        # PE p-state warm-up: the PE needs ~3us of continuous work to reach
        # full clock. Fill the input-DMA wait with dummy matmuls on zeroed
        # scratch so the real stream starts warm instead of ramping.
        warm = singles.tile([128, 512], FP8)
        nc.gpsimd.memset(warm[:], 0)
        wps = ctx.enter_context(tc.tile_pool(name="warmps", bufs=1, space="PSUM"))
        wp = wps.tile([128, 512], F32)
        for _ in range(11):
            nc.tensor.matmul(
                wp[:],
                lhsT=warm[:, :128],
                rhs=warm[:],
                start=True,
                stop=True,
                skip_group_check=True,
            )

        t = 0
        opair = 0
        for gsz in groups:
            rs = spool.tile([128, gsz], F32)
            nums = []
            for j in range(gsz):
                if t % 4 == 0:  # 4 tiles per input DMA: 2KB per partition line
                    bt_t = bpool.tile([128, 4, NCH, 128], FP8)
                    nc.sync.dma_start(out=bt_t[:], in_=bt[:, ts(t // 4, 4)])
                w = t % 4
                ps = psum.tile([128, K], F32)  # 2 banks; each matmul hits one
                # interleave the two kh accumulation groups so every
                # LDWEIGHTS hides under the previous matmul's stream
                for c in range(NCH):
                    for kh in range(2):
                        nc.tensor.matmul(
                            ps[:, ts(kh, 512)],
                            lhsT=bt_t[:, w, c],
                            rhs=ct_sb[:, c, ts(kh, 512)],
                            start=(c == 0),
                            stop=(c == NCH - 1),
                            skip_group_check=True,
                        )
                num = npool.tile([128, K], FP16)
                _act_reciprocal(nc, num[:], ps[:], sc_sb[:, t : t + 1], rs[:, j : j + 1])
                nums.append(num)
                t += 1
            inv = spool.tile([128, gsz], F32)
            nc.vector.reciprocal(out=inv[:], in_=rs[:])
            for pj in range(gsz // 2):  # paired output DMAs: 4KB/partition
                o2 = opool.tile([128, 2, K], FP16)
                for w2 in range(2):
                    j = 2 * pj + w2
                    nc.vector.tensor_scalar_mul(
                        o2[:, w2], nums[j][:], inv[:, j : j + 1]
                    )
                eng = nc.gpsimd if opair % 2 == 0 else nc.sync
                eng.dma_start(out=out[opair].rearrange("a b c -> b a c"), in_=o2[:])
                opair += 1
    nc.finalize()
    return nc


_NC_CACHE = None


def _get_nc():
    global _NC_CACHE
    if _NC_CACHE is None:
        _NC_CACHE = build_bass()
    return _NC_CACHE


def prepare_inputs(batch: np.ndarray, cluster_centers: np.ndarray):
    """Host-side shard + layout. Returns in_maps for run_bass_kernel_spmd."""
    assert batch.shape == (N, D) and cluster_centers.shape == (K, D)
    b32 = batch.astype(np.float32, copy=False)
    c32 = cluster_centers.astype(np.float32, copy=False)
    xsq = np.einsum("nd,nd->n", b32, b32)  # [N]
    csq = np.einsum("kd,kd->k", c32, c32)  # [K]

    # ct[p, c, k] = -2 * centers[k, c*128+p]; stolen row 511 carries csq_k
    cmod = -2.0 * c32  # [K, D]
    ctf = cmod.T.reshape(NCH, 128, K).transpose(1, 0, 2)  # [p, c, k]
    ctf = np.ascontiguousarray(ctf, dtype=NP_FP8)
    ctf[127, NCH - 1, :] = csq.astype(NP_FP8)

    in_maps = []
    for i in range(NCORES):
        shard = b32[i * NS : (i + 1) * NS]
        # bt[p, t, c, j] = shard[t*128+j, c*128+p]; stolen row 511 = 1.0
        bts = shard.reshape(NT, 128, NCH, 128).transpose(3, 0, 2, 1)
        bts = np.ascontiguousarray(bts, dtype=NP_FP8)
        bts[127, :, NCH - 1, :] = 1.0
        # sc[p, t] = 1/(1 + xsq[t*128+p]), exact f32
        scs = 1.0 / (
            1.0 + xsq[i * NS : (i + 1) * NS].reshape(NT, 128).T
        )
        scs = np.ascontiguousarray(scs, dtype=np.float32)
        in_maps.append({"bt": bts, "ct": ctf, "sc": scs})
    return in_maps


def kernel(batch: np.ndarray, cluster_centers: np.ndarray, _trace=False) -> np.ndarray:
    nc = _get_nc()
    in_maps = prepare_inputs(batch, cluster_centers)
    res = run_bass_kernel_spmd(nc, in_maps, list(range(NCORES)), trace=_trace)
    out = np.concatenate(
        [
            res.results[i]["out"].reshape(NS, K).astype(np.float32)
            for i in range(NCORES)
        ],
        axis=0,
    )
    if _trace:
        return out, res
    return out


# revision 21
# speedup vs baseline: 1.0334x; 1.0103x over previous
"""ClusterAssignment (Student-t / vq codebook soft-assignment) Trainium2 kernel.

Math (ALPHA=1 => power=1):
    ns[n,k]  = max(||x_n - c_k||^2, 0) = ||x||^2 + ||c||^2 - 2 x.c   (>= ~430 here, relu moot)
    num[n,k] = 1 / (1 + ns[n,k])
    out[n,k] = num[n,k] / sum_k num[n,k]

Strategy: data-parallel over 8 NeuronCores (batch N=65536 -> 8192 rows/core,
centers replicated; no collectives). Per 128-row batch tile, the PE computes
P[n,k] = csq_k - 2 x.c in a 2-bank PSUM tile [128,1024] with 8 fp8 matmuls
(4 contraction chunks x 2 K-halves) and NOTHING else -- both norm terms ride
for free:

  - csq_k: contraction row 511 is stolen (x_511 dropped, ~1.8e-4 noise vs a
    2e-2 budget): bt row = 1.0, ct row = csq_k. No aug matmuls.
  - xsq_n: folded into the ScalarE ACT *scale* operand (exact f32):
        num'[n,k] = 1/(P*sc_n + 1) = b_n * num[n,k],  sc_n = 1/b_n = 1/(1+xsq_n)
    The b_n factor cancels in the final normalization, since the row-sum
    accumulator also scales by b_n. ACT input sits in [0.985, 1.019], the
    best-conditioned spot for the reciprocal table.

This puts the PE at the TRN2 roofline for this op: 4096 streamed columns per
tile, 1 column/cycle (DoubleRow/DoublePixel modes were measured/confirmed to
give no net stream speedup on this silicon), ~112us of gapless matmul per
core; everything else hides under it.

Epilogue per tile: ONE ScalarE Reciprocal pass (scale=sc_n, bias=1.0) reads
the full [128,1024] PSUM tile, writing num' as fp16 AND the f32 row-sum via
the ACT accumulator. Row-sum reciprocals are batched 8 tiles per DVE
instruction (4 for the last tiles, shortening the tail); out = num' * inv'
on DVE (fp16 4x mode); paired output DMAs (2 tiles -> one 4KB/partition
transfer) alternate between the Sync HWDGE queue and the otherwise-idle
GpSimd SWDGE queue so the output drain parallelizes. Host upcasts
fp16 -> f32.
"""

import sys

sys.path.insert(0, "/opt/trn_rl_repo")

from contextlib import ExitStack

import ml_dtypes
import numpy as np

import concourse.bass as bass
import concourse.mybir as mybir
import concourse.tile as tile
from concourse import bacc
from concourse.bass import ts
from concourse.bass_utils import run_bass_kernel_spmd

N, K, D = 65536, 512 * 2, 512  # K=1024
NCORES = 8
NS = N // NCORES  # 8192 rows per core
NT = NS // 128  # 64 tiles per core
NCH = D // 128  # 4 contraction chunks of 128
BF16 = mybir.dt.bfloat16
F32 = mybir.dt.float32
FP16 = mybir.dt.float16
FP8 = mybir.dt.float8e4  # e4m3 (TRN variant: max normal 240 -- our data is <6)
NP_FP8 = ml_dtypes.float8_e4m3


def _act_reciprocal(nc, out_ap, in_ap, scale_ap, accum_ap):
    """ScalarE activation out = 1/(in_*scale + 1) with row-sum accumulator.

    bass's activation() refuses ActivationFunctionType.Reciprocal because of
    known accuracy issues in the general case; on this kernel's input range
    ([~0.985, ~1.019] after the scale/bias affine) the error is at the fp16
    output rounding floor and the f32 accumulator is accurate to ~2e-6, so we
    emit the instruction directly. scale is a per-partition [128,1] f32 AP.
    """
    eng = nc.scalar
    ins = [
        eng.lower_ap(in_ap),
        mybir.ImmediateValue(dtype=F32, value=1.0),  # bias
        eng.lower_ap(scale_ap),  # scale = 1/(1+xsq_n)
        mybir.ImmediateValue(dtype=F32, value=0.0),  # alpha
    ]
    outs = [eng.lower_ap(out_ap), eng.lower_ap(accum_ap)]
    return eng.add_instruction(
        mybir.InstActivation(
            name=nc.get_next_instruction_name(),
            func=mybir.ActivationFunctionType.Reciprocal,
            ins=ins,
            outs=outs,
        )
    )


def build_bass():
    nc = bacc.Bacc("TRN2", target_bir_lowering=False, debug=False)
    bt = nc.declare_dram_parameter("bt", [128, NT, NCH, 128], FP8, isOutput=False)
    ct = nc.declare_dram_parameter("ct", [128, NCH, K], FP8, isOutput=False)
    sc = nc.declare_dram_parameter("sc", [128, NT], F32, isOutput=False)
    # out[u, w, p, k] = row u*256 + w*128 + p -> host reshapes to [NS, K]
    out = nc.declare_dram_parameter("out", [NT // 2, 2, 128, K], FP16, isOutput=True)

    # tiles per row-sum reciprocal batch: one DVE inv instruction per group;
    # smaller final groups release the last output DMAs sooner
    groups = [8] * 6 + [4, 4, 2, 2, 2, 2]
    assert sum(groups) == NT

    with tile.TileContext(nc) as tc, ExitStack() as ctx:
        singles = ctx.enter_context(tc.tile_pool(name="singles", bufs=1))
        bpool = ctx.enter_context(tc.tile_pool(name="bt", bufs=3))
        npool = ctx.enter_context(tc.tile_pool(name="num", bufs=18))
        opool = ctx.enter_context(tc.tile_pool(name="outp", bufs=4))
        spool = ctx.enter_context(tc.tile_pool(name="small", bufs=6))
        psum = ctx.enter_context(tc.tile_pool(name="psum", bufs=3, space="PSUM"))

        # ct per-chunk on the Sync HWDGE queue (the queue fans out to ~15 DMA
        # engines): the first matmul only needs chunk 0 + bt tile 0, so the
        # stream starts ~2us earlier than with one monolithic ct transfer.
        ct_sb = singles.tile([128, NCH, K], FP8)
        sc_sb = singles.tile([128, NT], F32)
        bt0 = bpool.tile([128, 4, NCH, 128], FP8)
        nc.sync.dma_start(out=ct_sb[:, 0], in_=ct[:, 0])
        nc.sync.dma_start(out=bt0[:, 0:1], in_=bt[:, 0:1])
        for c in range(1, NCH):
            nc.sync.dma_start(out=ct_sb[:, c], in_=ct[:, c])
        nc.sync.dma_start(out=bt0[:, 1:4], in_=bt[:, 1:4])
        nc.sync.dma_start(out=sc_sb[:], in_=sc[:])

        # PE p-state warm-up: the PE needs ~3us of continuous work to reach
        # full clock. Fill the input-DMA wait with dummy matmuls on zeroed
        # scratch so the real stream starts warm instead of ramping.
        warm = singles.tile([128, 512], FP8)
        nc.gpsimd.memset(warm[:], 0)
        wps = ctx.enter_context(tc.tile_pool(name="warmps", bufs=1, space="PSUM"))
        wp = wps.tile([128, 512], F32)
        for _ in range(7):
            nc.tensor.matmul(
                wp[:],
                lhsT=warm[:, :128],
                rhs=warm[:],
                start=True,
                stop=True,
                skip_group_check=True,
            )

        t = 0
        opair = 0
        for gsz in groups:
            rs = spool.tile([128, gsz], F32)
            nums = []
            for j in range(gsz):
                if t % 4 == 0:  # 4 tiles per input DMA: 2KB per partition line
                    if t == 0:
                        bt_t = bt0  # prefetched above, tile 0 first
                    else:
                        bt_t = bpool.tile([128, 4, NCH, 128], FP8)
                        nc.sync.dma_start(out=bt_t[:], in_=bt[:, ts(t // 4, 4)])
                w = t % 4
                ps = psum.tile([128, K], F32)  # 2 banks; each matmul hits one
                # interleave the two kh accumulation groups so every
                # LDWEIGHTS hides under the previous matmul's stream
                for c in range(NCH):
                    for kh in range(2):
                        nc.tensor.matmul(
                            ps[:, ts(kh, 512)],
                            lhsT=bt_t[:, w, c],
                            rhs=ct_sb[:, c, ts(kh, 512)],
                            start=(c == 0),
                            stop=(c == NCH - 1),
                            skip_group_check=True,
                        )
                num = npool.tile([128, K], FP16)
                _act_reciprocal(nc, num[:], ps[:], sc_sb[:, t : t + 1], rs[:, j : j + 1])
                nums.append(num)
                t += 1
            inv = spool.tile([128, gsz], F32)
            nc.vector.reciprocal(out=inv[:], in_=rs[:])
            if gsz > 2:
                for pj in range(gsz // 2):  # paired output DMAs: 4KB/partition
                    o2 = opool.tile([128, 2, K], FP16)
                    for w2 in range(2):
                        j = 2 * pj + w2
                        nc.vector.tensor_scalar_mul(
                            o2[:, w2], nums[j][:], inv[:, j : j + 1]
                        )
                    eng = nc.gpsimd if opair % 2 == 0 else nc.sync
                    eng.dma_start(
                        out=out[opair].rearrange("a b c -> b a c"), in_=o2[:]
                    )
                    opair += 1
            else:
                # final tiles: unpaired 2KB/partition DMAs on alternating
                # queues halve the last transfer's latency
                for j2 in range(2):
                    o1 = opool.tile([128, K], FP16)
                    nc.vector.tensor_scalar_mul(
                        o1[:], nums[j2][:], inv[:, j2 : j2 + 1]
                    )
                    eng = nc.gpsimd if j2 == 0 else nc.sync
                    eng.dma_start(out=out[opair, j2], in_=o1[:])
                opair += 1
    nc.finalize()
    return nc


_NC_CACHE = None


def _get_nc():
    global _NC_CACHE
    if _NC_CACHE is None:
        _NC_CACHE = build_bass()
    return _NC_CACHE


def prepare_inputs(batch: np.ndarray, cluster_centers: np.ndarray):
    """Host-side shard + layout. Returns in_maps for run_bass_kernel_spmd."""
    assert batch.shape == (N, D) and cluster_centers.shape == (K, D)
    b32 = batch.astype(np.float32, copy=False)
    c32 = cluster_centers.astype(np.float32, copy=False)
    xsq = np.einsum("nd,nd->n", b32, b32)  # [N]
    csq = np.einsum("kd,kd->k", c32, c32)  # [K]

    # ct[p, c, k] = -2 * centers[k, c*128+p]; stolen row 511 carries csq_k
    cmod = -2.0 * c32  # [K, D]
    ctf = cmod.T.reshape(NCH, 128, K).transpose(1, 0, 2)  # [p, c, k]
    ctf = np.ascontiguousarray(ctf, dtype=NP_FP8)
    ctf[127, NCH - 1, :] = csq.astype(NP_FP8)

    in_maps = []
    for i in range(NCORES):
        shard = b32[i * NS : (i + 1) * NS]
        # bt[p, t, c, j] = shard[t*128+j, c*128+p]; stolen row 511 = 1.0
        bts = shard.reshape(NT, 128, NCH, 128).transpose(3, 0, 2, 1)
        bts = np.ascontiguousarray(bts, dtype=NP_FP8)
        bts[127, :, NCH - 1, :] = 1.0
        # sc[p, t] = 1/(1 + xsq[t*128+p]), exact f32
        scs = 1.0 / (
            1.0 + xsq[i * NS : (i + 1) * NS].reshape(NT, 128).T
        )
        scs = np.ascontiguousarray(scs, dtype=np.float32)
        in_maps.append({"bt": bts, "ct": ctf, "sc": scs})
    return in_maps


def kernel(batch: np.ndarray, cluster_centers: np.ndarray, _trace=False) -> np.ndarray:
    nc = _get_nc()
    in_maps = prepare_inputs(batch, cluster_centers)
    res = run_bass_kernel_spmd(nc, in_maps, list(range(NCORES)), trace=_trace)
    out = np.concatenate(
        [
            res.results[i]["out"].reshape(NS, K).astype(np.float32)
            for i in range(NCORES)
        ],
        axis=0,
    )
    if _trace:
        return out, res
    return out
